# revision 19
# baseline (speedup 1.0000x reference)
"""ALiBi causal attention on 8 Trainium2 NeuronCores.

Sharding: tensor-parallel over heads (2 heads/core) for QKV projection and
attention; two batch-split AllToAlls redistribute the (normalized,
transposed) attention outputs so each core owns 256 tokens of each batch
for the output projection. The b0 AllToAll and b0 output projection
overlap with b1's attention compute.

Layout choices (all chosen to avoid on-chip transposes):
  - x is passed host-transposed as xT [D=1024, B*T=4096] in bf16.
  - Q/K are produced in "head-transposed" layout [head_dim, tokens] and
    augmented with one extra contraction row so that the per-query ALiBi
    term -slope*i rides the score matmul (exactly cancelled by softmax,
    so bf16 rounding of it is harmless).
  - Scores are computed transposed: ST[k, q] = K'.T-block @ Q', so the
    softmax reduction (over k) aligns with the AV matmul contraction and
    the denominator falls out of a ones-column appended to V.
  - exp via ScalarE with per-partition bias slope*j in exact f32.
  - Causal masking: only the diagonal-intersecting k-block per q-tile
    needs a 128x128 triangular min-clamp; fully-masked columns are never
    computed or streamed.

Tiles are deliberately small/chunked (xT per [k,512-token] block, Q/K per
[head, 512-token] chunk, V per [token-block]) because Tile's dependency
tracking is per-tile: chunking lets attention start while later
projections still run, and projections start after the first DMA chunk.
DMA queues: xT streams on the sync queue; weights/constants go on the
scalar queue so they don't delay the first projection matmuls.
"""

import sys

if "/opt/trn_rl_repo" not in sys.path:
    sys.path.insert(0, "/opt/trn_rl_repo")

import numpy as np
import ml_dtypes

import concourse.bass as bass
import concourse.bacc as bacc
import concourse.tile as tile
import concourse.mybir as mybir
from concourse import bass_utils

BF16 = mybir.dt.bfloat16
F32 = mybir.dt.float32
NPBF16 = ml_dtypes.bfloat16

B, T, D = 2, 2048, 1024
H, HD = 16, 64
NC = 8
HPC = H // NC          # heads per core = 2
TOK = B * T            # 4096
TPC = TOK // NC        # tokens per core after a2a = 512 (256 per batch)
NKB = T // 128         # 16 k-blocks per sequence
NQT = T // 512         # 4 q-tiles per sequence
NTC = TOK // 512       # 8 token-chunks of 512
KAUG = HD + 1          # 65: head_dim + 1 aug row

_COMPILED = None


def _build():
    nc = bacc.Bacc("TRN2", target_bir_lowering=False, debug=False, num_devices=NC)

    xT_d = nc.dram_tensor("xT", [D, TOK], BF16, kind="ExternalInput")
    wq_d = nc.dram_tensor("wq", [D, 128], BF16, kind="ExternalInput")
    wk_d = nc.dram_tensor("wk", [D, 128], BF16, kind="ExternalInput")
    wv_d = nc.dram_tensor("wv", [D, 128], BF16, kind="ExternalInput")
    wo_d = nc.dram_tensor("wo", [D, D], BF16, kind="ExternalInput")
    aug_d = nc.dram_tensor("aug", [HPC + 1, T], BF16, kind="ExternalInput")
    kbias_d = nc.dram_tensor("kbias", [128, HPC * NKB], F32, kind="ExternalInput")
    cap_d = nc.dram_tensor("cap", [128, 128], F32, kind="ExternalInput")
    ind_d = nc.dram_tensor("ind", [1, 256], BF16, kind="ExternalInput")
    out_d = nc.dram_tensor("out", [TPC, D], F32, kind="ExternalOutput")
    ccin = [
        nc.dram_tensor(f"ccin{b}", [NC * 128, TPC // B], BF16, kind="Internal")
        for b in range(B)
    ]
    ccout = [
        nc.dram_tensor(f"ccout{b}", [NC * 128, TPC // B], BF16, kind="Internal")
        for b in range(B)
    ]

    with tile.TileContext(nc) as tc:
        with (
            tc.tile_pool(name="const", bufs=1) as cpool,
            tc.tile_pool(name="work", bufs=1) as wpool,
            tc.tile_pool(name="ps", bufs=4, space="PSUM") as ps,
            tc.tile_pool(name="psot", bufs=4, space="PSUM") as psot,
        ):
            # ---- xT: one tile per (k-chunk, token-chunk), sync queue --
            xt = [[None] * NTC for _ in range(8)]
            for tc8 in range(NTC):
                for k in range(8):
                    t_ = cpool.tile([128, 512], BF16, name=f"xt{k}_{tc8}", tag=f"xt{k}_{tc8}")
                    nc.sync.dma_start(t_[:], xT_d[128 * k : 128 * (k + 1), 512 * tc8 : 512 * (tc8 + 1)])
                    xt[k][tc8] = t_

            # ---- weights + constants on the scalar DMA queue ----------
            wq_t = cpool.tile([128, D], BF16, name="wq_t", tag="wq_t")
            wk_t = cpool.tile([128, D], BF16, name="wk_t", tag="wk_t")
            wv_t = cpool.tile([128, D], BF16, name="wv_t", tag="wv_t")
            for k in range(8):
                nc.scalar.dma_start(wq_t[:, 128 * k : 128 * (k + 1)], wq_d[128 * k : 128 * (k + 1), :])
                nc.scalar.dma_start(wk_t[:, 128 * k : 128 * (k + 1)], wk_d[128 * k : 128 * (k + 1), :])
                nc.scalar.dma_start(wv_t[:, 128 * k : 128 * (k + 1)], wv_d[128 * k : 128 * (k + 1), :])
            kbias_t = cpool.tile([128, HPC * NKB], F32, name="kbias_t", tag="kbias_t")
            nc.scalar.dma_start(kbias_t[:], kbias_d[:])
            cap_t = cpool.tile([128, 128], F32, name="cap_t", tag="cap_t")
            nc.scalar.dma_start(cap_t[:], cap_d[:])
            ind_t = cpool.tile([1, 256], BF16, name="ind_t", tag="ind_t")
            nc.scalar.dma_start(ind_t[:], ind_d[:])

            # QTa/KTa: per (b, hl, chunk-of-512): [65, 512]; row 64 = aug.
            qta = [[[None] * NQT for _ in range(HPC)] for _ in range(B)]
            kta = [[[None] * NQT for _ in range(HPC)] for _ in range(B)]
            for b in range(B):
                for hl in range(HPC):
                    for c in range(NQT):
                        q_ = cpool.tile([KAUG, 512], BF16, name=f"qta{b}{hl}{c}", tag=f"qta{b}{hl}{c}")
                        k_ = cpool.tile([KAUG, 512], BF16, name=f"kta{b}{hl}{c}", tag=f"kta{b}{hl}{c}")
                        nc.scalar.dma_start(q_[64:65, :], aug_d[hl : hl + 1, 512 * c : 512 * (c + 1)])
                        nc.scalar.dma_start(k_[64:65, :], aug_d[HPC : HPC + 1, 512 * c : 512 * (c + 1)])
                        qta[b][hl][c] = q_
                        kta[b][hl][c] = k_
            # V: per (b, k-block): [128, 130]: 64 cols head A, ones col,
            # 64 cols head B, ones col.
            vt = [[None] * NKB for _ in range(B)]
            for b in range(B):
                for kb in range(NKB):
                    v_ = cpool.tile([128, 130], BF16, name=f"v{b}_{kb}", tag=f"v{b}_{kb}")
                    nc.vector.memset(v_.rearrange("p (a c) -> p a c", c=65)[:, :, 64], 1.0)
                    vt[b][kb] = v_

            # ---- PE warm-up: dependency-free matmuls on scratch data so
            # the HAM clock gate reaches 8/8 before the real work arrives.
            warm_in = cpool.tile([128, 512], BF16, name="warm_in", tag="warm_in")
            nc.gpsimd.memset(warm_in[:], 0.0)
            for _ in range(30):
                wps = psot.tile([128, 512], F32, name="wps", tag="otv")
                nc.tensor.matmul(wps[:], warm_in[:, 0:128], warm_in[:], start=True, stop=True)

            # ---- phase 1: QKV projections (chunk-interleaved) ---------
            def qkv_chunk(tc8):
                b, cq = tc8 // NQT, tc8 % NQT
                for w_t, dsts in ((wq_t, qta), (wk_t, kta)):
                    pp = ps.tile([128, 512], F32, name="pp", tag="mm512")
                    for k in range(8):
                        nc.tensor.matmul(
                            pp[:],
                            w_t[:, 128 * k : 128 * (k + 1)],
                            xt[k][tc8][:],
                            start=(k == 0),
                            stop=(k == 7),
                        )
                    nc.scalar.copy(dsts[b][0][cq][0:64, :], pp[0:64, :])
                    nc.scalar.copy(dsts[b][1][cq][0:64, :], pp[64:128, :])
                for j in range(4):
                    kb = 4 * cq + j
                    pv = psot.tile([128, 128], F32, name="pv", tag="otv")
                    for k in range(8):
                        nc.tensor.matmul(
                            pv[:],
                            xt[k][tc8][:, 128 * j : 128 * (j + 1)],
                            wv_t[:, 128 * k : 128 * (k + 1)],
                            start=(k == 0),
                            stop=(k == 7),
                        )
                    nc.vector.tensor_copy(vt[b][kb][:, 0:64], pv[:, 0:64])
                    nc.vector.tensor_copy(vt[b][kb][:, 65:129], pv[:, 64:128])

            # ---- phase 2: attention for one (b, q-tile) ---------------
            def attn_group(b, qt):
                nkb = 4 * qt + 4
                ots = [
                    psot.tile([KAUG, 512], F32, name="ot", tag="otv")
                    for _ in range(HPC)
                ]
                # Software pipelining: the AV matmul for k-block kb is
                # emitted one block behind the score matmul, so the static
                # PE stream never waits on the min->exp chain of the block
                # it just produced.
                pend = []
                for kb in range(nkb + 1):
                    if kb < nkb:
                        off = max(0, 128 * (kb - 4 * qt))
                        exs = []
                        for hl in range(HPC):
                            sc = ps.tile([128, 512], F32, name="sc", tag="mm512")
                            nc.tensor.matmul(
                                sc[:, off:512],
                                kta[b][hl][kb // 4][:, 128 * (kb % 4) : 128 * (kb % 4 + 1)],
                                qta[b][hl][qt][:, off:512],
                                start=True,
                                stop=True,
                            )
                            if kb >= 4 * qt:
                                nc.vector.tensor_tensor(
                                    sc[:, off : off + 128],
                                    sc[:, off : off + 128],
                                    cap_t[:],
                                    mybir.AluOpType.min,
                                )
                            ex = wpool.tile([128, 512], BF16, name="ex", tag="ex", bufs=6)
                            nc.scalar.activation(
                                ex[:, off:512],
                                sc[:, off:512],
                                mybir.ActivationFunctionType.Exp,
                                bias=kbias_t[:, NKB * hl + kb : NKB * hl + kb + 1],
                                scale=0.125,
                            )
                            exs.append(ex)
                        pend.append((kb, off, exs))
                    if kb >= 1:
                        pkb, poff, pexs = pend.pop(0)
                        for hl in range(HPC):
                            nc.tensor.matmul(
                                ots[hl][:, poff:512],
                                vt[b][pkb][:, 65 * hl : 65 * hl + 65],
                                pexs[hl][:, poff:512],
                                start=(pkb == 0),
                                stop=(pkb == nkb - 1),
                            )
                dena = wpool.tile([1, 512], BF16, name="dena", tag="dena", bufs=2)
                denb = wpool.tile([1, 512], BF16, name="denb", tag="denb", bufs=2)
                nc.vector.tensor_copy(dena[:], ots[0][64:65, :])
                nc.vector.tensor_copy(denb[:], ots[1][64:65, :])
                bc = ps.tile([128, 512], F32, name="bc", tag="mm512")
                nc.tensor.matmul(bc[:], ind_t[:, 0:128], dena[:], start=True, stop=False)
                nc.tensor.matmul(bc[:], ind_t[:, 128:256], denb[:], start=False, stop=True)
                bcs = wpool.tile([128, 512], F32, name="bcs", tag="bcs", bufs=2)
                nc.vector.tensor_copy(bcs[:], bc[:])
                bci = wpool.tile([128, 512], F32, name="bci", tag="bci", bufs=2)
                nc.vector.reciprocal_approx_fast(bci[:], bcs[:])
                otn = wpool.tile([128, 512], BF16, name="otn", tag="otn", bufs=3)
                nc.vector.tensor_tensor(
                    otn[0:64, :], ots[0][0:64, :], bci[0:64, :], mybir.AluOpType.mult
                )
                nc.vector.tensor_tensor(
                    otn[64:128, :], ots[1][0:64, :], bci[64:128, :], mybir.AluOpType.mult
                )
                # two destination blocks of 256 tokens each
                for half in range(2):
                    blk = 2 * qt + half
                    nc.sync.dma_start(
                        ccin[b][128 * blk : 128 * (blk + 1), :],
                        otn[:, 256 * half : 256 * (half + 1)],
                    )

            # ---- phase 4: output projection for one batch -------------
            at = [[None] * 8 for _ in range(B)]

            def yrecv(b):
                for k in range(8):
                    a_ = cpool.tile([128, TPC // B], BF16, name=f"at{b}_{k}", tag=f"at{b}_{k}")
                    nc.sync.dma_start(a_[:], ccout[b][128 * k : 128 * (k + 1), :])
                    at[b][k] = a_

            def ypiece(b, tb, n):
                yp = ps.tile([128, 512], F32, name="yp", tag="mm512")
                for k in range(8):
                    nc.tensor.matmul(
                        yp[:],
                        at[b][k][:, 128 * tb : 128 * (tb + 1)],
                        wo_t[:, D * k + 512 * n : D * k + 512 * (n + 1)],
                        start=(k == 0),
                        stop=(k == 7),
                    )
                ys = wpool.tile([128, 512], F32, name="ys", tag="ys", bufs=2)
                nc.vector.tensor_copy(ys[:], yp[:])
                nc.sync.dma_start(
                    out_d[256 * b + 128 * tb : 256 * b + 128 * (tb + 1), 512 * n : 512 * (n + 1)],
                    ys[:],
                )

            def yproj(b):
                yrecv(b)
                for tb in range(2):
                    for n in range(D // 512):
                        ypiece(b, tb, n)

            # ---- schedule -------------------------------------------
            for tc8 in range(NTC):
                qkv_chunk(tc8)

            # wo arrives during attention on the scalar queue
            wo_t = cpool.tile([128, 8 * D], BF16, name="wo_t", tag="wo_t")
            for k in range(8):
                nc.scalar.dma_start(wo_t[:, D * k : D * (k + 1)], wo_d[128 * k : 128 * (k + 1), :])

            for qt in range(NQT):
                attn_group(0, qt)
            nc.gpsimd.collective_compute(
                "AllToAll",
                mybir.AluOpType.bypass,
                replica_groups=[list(range(NC))],
                ins=[ccin[0][:]],
                outs=[ccout[0][:]],
            )
            # a2a#0 runs on the collective engine while the PE continues
            # with b1 attention; Y(b0) goes after b1 attention so the static
            # PE stream never stalls on the collective mid-attention, and
            # a2a#1 overlaps the Y(b0) matmuls.
            yrecv(0)
            for qt in range(NQT):
                attn_group(1, qt)
            nc.gpsimd.collective_compute(
                "AllToAll",
                mybir.AluOpType.bypass,
                replica_groups=[list(range(NC))],
                ins=[ccin[1][:]],
                outs=[ccout[1][:]],
            )
            for tb in range(2):
                for n in range(D // 512):
                    ypiece(0, tb, n)
            yproj(1)

    nc.compile()
    return nc


def _host_inputs(x, Wq, Wk, Wv, Wo):
    x = np.asarray(x, dtype=np.float32)
    Wq, Wk, Wv, Wo = (np.asarray(w, dtype=np.float32) for w in (Wq, Wk, Wv, Wo))
    toks = x.reshape(TOK, D)
    xT = np.ascontiguousarray(toks.T).astype(NPBF16)
    wo_t = np.ascontiguousarray(Wo.T).astype(NPBF16)
    base = 2.0 ** (-8.0 / H)

    cap = np.where(
        np.arange(128)[:, None] <= np.arange(128)[None, :], 3.0e38, -1.0e9
    ).astype(np.float32)
    ind = np.zeros((1, 256), dtype=NPBF16)
    ind[0, 0:64] = 1      # head-A indicator: bc rows 0:64 get denA
    ind[0, 192:256] = 1   # head-B indicator: bc rows 64:128 get denB
    pos_bf = np.arange(T, dtype=np.float32).astype(NPBF16).astype(np.float32)

    in_maps = []
    for c in range(NC):
        hs = slice(128 * c, 128 * (c + 1))
        aug = np.zeros((HPC + 1, T), dtype=NPBF16)
        aug[HPC] = 1
        kbias = np.zeros((128, HPC * NKB), dtype=np.float32)
        for hl in range(HPC):
            h = HPC * c + hl
            slope = base ** (h + 1)
            aug[hl] = (-8.0 * slope * pos_bf).astype(NPBF16)
            for kb in range(NKB):
                kbias[:, NKB * hl + kb] = slope * (128 * kb + np.arange(128))
        in_maps.append(
            {
                "xT": xT,
                "wq": np.ascontiguousarray(Wq[hs, :].T).astype(NPBF16),
                "wk": np.ascontiguousarray(Wk[hs, :].T).astype(NPBF16),
                "wv": np.ascontiguousarray(Wv[hs, :].T).astype(NPBF16),
                "wo": wo_t,
                "aug": aug,
                "kbias": kbias,
                "cap": cap,
                "ind": ind,
            }
        )
    return in_maps


def get_compiled():
    global _COMPILED
    if _COMPILED is None:
        _COMPILED = _build()
    return _COMPILED


def run(x, Wq, Wk, Wv, Wo, trace=False, **trace_kwargs):
    nc = get_compiled()
    in_maps = _host_inputs(x, Wq, Wk, Wv, Wo)
    res = bass_utils.run_bass_kernel_spmd(
        nc, in_maps, core_ids=list(range(NC)), trace=trace, **trace_kwargs
    )
    full = np.empty((TOK, D), dtype=np.float32)
    half = TPC // B  # 256
    for c in range(NC):
        o = res.results[c]["out"]
        full[half * c : half * (c + 1), :] = o[0:half]
        full[T + half * c : T + half * (c + 1), :] = o[half : 2 * half]
    return full.reshape(B, T, D), res


def kernel(x, Wq, Wk, Wv, Wo):
    out, _ = run(x, Wq, Wk, Wv, Wo)
    return out


# revision 39
# speedup vs baseline: 1.3261x; 1.3261x over previous
"""ALiBi causal attention on 8 Trainium2 NeuronCores.

Sharding: tensor-parallel over heads (2 heads/core). Core c owns global
heads c (steep slope) and 8+c (shallow slope) so that ALiBi block-skipping
gives every core the same instruction stream: steep heads only attend to
the ~8 k-blocks nearest the diagonal (older blocks decay below e^-30 of
the max weight), shallow heads attend to everything. Two batch-split
AllToAlls redistribute the (normalized, transposed) attention outputs so
each core owns 256 tokens of each batch for the output projection; the
b0 AllToAll overlaps b1's attention compute.

Layout choices (all chosen to avoid on-chip transposes):
  - x is passed host-transposed as xT [D=1024, B*T=4096] in bf16.
  - Q/K are produced in "head-transposed" layout [head_dim, tokens] with
    THREE augmented contraction rows carrying the ALiBi bias through the
    score matmul exactly:
      row 64: K=1,      Q=-8*slope*bf16(i)   (per-query term; any rounding
              cancels in softmax, so bf16 is safe)
      row 65: K=kb,     Q=C   where C = bf16(1024*slope); kb<=15 is exact
              in bf16 so C*kb accumulates exactly in f32
      row 66: K=kb,     Q=Dr  where Dr = bf16(1024*slope - C) mops up the
              rounding of C (double-bf16 trick)
    leaving only slope*p (p = partition index, exact f32) for the ScalarE
    exp bias -- which is then the same for every k-block of a head, so
    one exp instruction spans a PAIR of k-blocks (halves ACT op count;
    ACT exp throughput is the attention-phase bottleneck).
  - Scores are computed transposed: ST[k, q] = K'.T-block @ Q', so the
    softmax reduction (over k) aligns with the AV matmul contraction and
    the denominator falls out of a ones-column appended to V.
  - Causal masking: only the diagonal-intersecting k-block per q-tile
    needs a 128x128 triangular min-clamp; fully-masked columns are never
    computed or streamed.

Tiles are deliberately small/chunked (xT per [k,512-token] block, Q/K per
[head, 512-token] chunk, V per [token-block]) because Tile's dependency
tracking is per-tile: projection chunk i feeds attention group i 1:1 in
the schedule. DMA queues: xT streams on the sync queue; weights and
constants go on the scalar queue so they don't delay the first matmuls.
"""

import sys

if "/opt/trn_rl_repo" not in sys.path:
    sys.path.insert(0, "/opt/trn_rl_repo")

import numpy as np
import ml_dtypes

import concourse.bass as bass
import concourse.bacc as bacc
import concourse.tile as tile
import concourse.mybir as mybir
from concourse import bass_utils

BF16 = mybir.dt.bfloat16
F32 = mybir.dt.float32
NPBF16 = ml_dtypes.bfloat16

B, T, D = 2, 2048, 1024
H, HD = 16, 64
NC = 8
HPC = H // NC          # heads per core = 2
TOK = B * T            # 4096
TPC = TOK // NC        # tokens per core after a2a = 512 (256 per batch)
NKB = T // 128         # 16 k-blocks per sequence
NQT = T // 512         # 4 q-tiles per sequence
KAUG = HD + 3          # 67: head_dim + 3 aug rows
MAXA = 8               # steep-head (slot A) k-block window per q-tile

_COMPILED = None


def _kept(hl, qt):
    """k-blocks computed for head-slot hl in q-tile qt (always even count,
    contiguous, ending at the diagonal block 4*qt+3)."""
    hi = 4 * qt + 4
    lo = max(0, hi - MAXA) if hl == 0 else 0
    return list(range(lo, hi))


def _build():
    nc = bacc.Bacc("TRN2", target_bir_lowering=False, debug=False, num_devices=NC)

    xT_d = nc.dram_tensor("xT", [D, TOK], BF16, kind="ExternalInput")
    wqkv_d = nc.dram_tensor("wqkv", [D, 384], BF16, kind="ExternalInput")
    wo_d = nc.dram_tensor("wo", [D, D], BF16, kind="ExternalInput")
    qaug_d = nc.dram_tensor("qaug", [HPC * 3, T], BF16, kind="ExternalInput")
    kaug_d = nc.dram_tensor("kaug", [3, T], BF16, kind="ExternalInput")
    kbias_d = nc.dram_tensor("kbias", [128, HPC], F32, kind="ExternalInput")
    cap_d = nc.dram_tensor("cap", [128, 128], F32, kind="ExternalInput")
    ind_d = nc.dram_tensor("ind", [1, 256], F32, kind="ExternalInput")
    out_d = nc.dram_tensor("out", [TPC, D], F32, kind="ExternalOutput")
    ccin = [
        nc.dram_tensor(f"ccin{b}", [NC * 128, TPC // B], BF16, kind="Internal")
        for b in range(B)
    ]
    ccout = [
        nc.dram_tensor(f"ccout{b}", [NC * 128, TPC // B], BF16, kind="Internal")
        for b in range(B)
    ]

    with tile.TileContext(nc) as tc:
        with (
            tc.tile_pool(name="const", bufs=1) as cpool,
            tc.tile_pool(name="work", bufs=1) as wpool,
            tc.tile_pool(name="pspair", bufs=2, space="PSUM") as pspair,
            tc.tile_pool(name="psot", bufs=2, space="PSUM") as psot,
            tc.tile_pool(name="ps", bufs=2, space="PSUM") as ps,
        ):
            # ---- sync queue: projection weights first, then xT chunks.
            # No DMA triggers ride the Scalar or Vector queues: a trigger
            # stalls its issuing compute engine when the HWDGE ring backs
            # up, and ScalarE owns the exp stream (the bottleneck).
            # one merged weight tile: chunk k occupies cols [384k, 384k+384)
            # as [wq_k | wk_k | wv_k]
            wqkv_t = cpool.tile([128, 8 * 384], BF16, name="wqkv_t", tag="wqkv_t")
            for k in range(8):
                nc.sync.dma_start(
                    wqkv_t[:, 384 * k : 384 * (k + 1)], wqkv_d[128 * k : 128 * (k + 1), :]
                )
            xt = [[None] * (TOK // 512) for _ in range(8)]
            for tc8 in range(TOK // 512):
                for k in range(8):
                    t_ = cpool.tile([128, 512], BF16, name=f"xt{k}_{tc8}", tag=f"xt{k}_{tc8}")
                    nc.sync.dma_start(t_[:], xT_d[128 * k : 128 * (k + 1), 512 * tc8 : 512 * (tc8 + 1)])
                    xt[k][tc8] = t_

            # ---- PE warm-up: dependency-free matmuls on scratch data so
            # the HAM clock gate reaches 8/8 before the real work arrives.
            warm_in = cpool.tile([128, 512], BF16, name="warm_in", tag="warm_in")
            nc.vector.memset(warm_in[:], 0.0)
            for _ in range(30):
                wps = psot.tile([128, 512], F32, name="wps", tag="otv")
                nc.tensor.matmul(wps[:], warm_in[:, 0:128], warm_in[:], start=True, stop=True)

            # ---- collective warm-up: a tiny AllToAll absorbs the
            # first-collective setup cost while the PE does projections.
            ccw_in = nc.dram_tensor("ccwin", [128, 16], BF16, kind="Internal")
            ccw_out = nc.dram_tensor("ccwout", [128, 16], BF16, kind="Internal")
            nc.gpsimd.dma_start(ccw_in[:], kaug_d[0:1, 0:2048].rearrange("a (p c) -> (a p) c", p=128))
            nc.gpsimd.collective_compute(
                "AllToAll",
                mybir.AluOpType.bypass,
                replica_groups=[list(range(NC))],
                ins=[ccw_in[:]],
                outs=[ccw_out[:]],
            )

            # ---- constants + aug rows on the (otherwise idle) GpSimd
            # queue, in consumption order.
            kbias_t = cpool.tile([128, HPC], F32, name="kbias_t", tag="kbias_t")
            nc.gpsimd.dma_start(kbias_t[:], kbias_d[:])
            cap_t = cpool.tile([128, 128], F32, name="cap_t", tag="cap_t")
            nc.gpsimd.dma_start(cap_t[:], cap_d[:])
            ind_t = cpool.tile([1, 256], F32, name="ind_t", tag="ind_t")
            nc.gpsimd.dma_start(ind_t[:], ind_d[:])

            qta = [[[None] * NQT for _ in range(HPC)] for _ in range(B)]
            kta = [[[None] * NQT for _ in range(HPC)] for _ in range(B)]
            for b in range(B):
                for hl in range(HPC):
                    for c in range(NQT):
                        q_ = cpool.tile([KAUG, 512], BF16, name=f"qta{b}{hl}{c}", tag=f"qta{b}{hl}{c}")
                        k_ = cpool.tile([KAUG, 512], BF16, name=f"kta{b}{hl}{c}", tag=f"kta{b}{hl}{c}")
                        qta[b][hl][c] = q_
                        kta[b][hl][c] = k_

            for b in range(B):
                for c in range(NQT):
                    for hl in range(HPC):
                        nc.gpsimd.dma_start(
                            qta[b][hl][c][64:67, :],
                            qaug_d[3 * hl : 3 * hl + 3, 512 * c : 512 * (c + 1)],
                        )
                        nc.gpsimd.dma_start(
                            kta[b][hl][c][64:67, :],
                            kaug_d[0:3, 512 * c : 512 * (c + 1)],
                        )
            # V: per (b, k-block): [128, 130]: 64 cols head A, ones col,
            # 64 cols head B, ones col.
            vt = [[None] * NKB for _ in range(B)]
            for b in range(B):
                for kb in range(NKB):
                    v_ = cpool.tile([128, 130], BF16, name=f"v{b}_{kb}", tag=f"v{b}_{kb}")
                    nc.vector.memset(v_.rearrange("p (a c) -> p a c", c=65)[:, :, 64], 1.0)
                    vt[b][kb] = v_

            # ---- phase 1: QKV projections (chunk-interleaved) ---------
            def qkv_chunk(tc8):
                b, cq = tc8 // NQT, tc8 % NQT
                for woff, dsts, eng in ((0, qta, "s"), (128, kta, "v")):
                    pp = ps.tile([128, 512], F32, name="pp", tag="mm512")
                    for k in range(8):
                        nc.tensor.matmul(
                            pp[:],
                            wqkv_t[:, 384 * k + woff : 384 * k + woff + 128],
                            xt[k][tc8][:],
                            start=(k == 0),
                            stop=(k == 7),
                        )
                    if eng == "s":
                        nc.scalar.copy(dsts[b][0][cq][0:64, :], pp[0:64, :])
                        nc.scalar.copy(dsts[b][1][cq][0:64, :], pp[64:128, :])
                    else:
                        nc.vector.tensor_copy(dsts[b][0][cq][0:64, :], pp[0:64, :])
                        nc.vector.tensor_copy(dsts[b][1][cq][0:64, :], pp[64:128, :])
                for j in range(4):
                    kb = 4 * cq + j
                    pv = ps.tile([128, 128], F32, name="pv", tag="mm512")
                    for k in range(8):
                        nc.tensor.matmul(
                            pv[:],
                            xt[k][tc8][:, 128 * j : 128 * (j + 1)],
                            wqkv_t[:, 384 * k + 256 : 384 * k + 384],
                            start=(k == 0),
                            stop=(k == 7),
                        )
                    nc.vector.tensor_copy(vt[b][kb][:, 0:64], pv[:, 0:64])
                    nc.vector.tensor_copy(vt[b][kb][:, 65:129], pv[:, 64:128])

            # ---- phase 2: attention for one (b, q-tile) ---------------
            def attn_group(b, qt):
                ots = []
                for hl in range(HPC):
                    ot = psot.tile([65, 512], F32, name="ot", tag="otv")
                    ots.append(ot)
                    kept = _kept(hl, qt)
                    pairs = [(kept[i], kept[i + 1]) for i in range(0, len(kept), 2)]
                    pend = []
                    for pi in range(len(pairs) + 1):
                        if pi < len(pairs):
                            kb0, kb1 = pairs[pi]
                            offs = [max(0, 128 * (kb - 4 * qt)) for kb in (kb0, kb1)]
                            pr = pspair.tile([128, 1024], F32, name="pr", tag="pair")
                            for s, (kb, off) in enumerate(zip((kb0, kb1), offs)):
                                nc.tensor.matmul(
                                    pr[:, 512 * s + off : 512 * (s + 1)],
                                    kta[b][hl][kb // 4][:, 128 * (kb % 4) : 128 * (kb % 4 + 1)],
                                    qta[b][hl][qt][:, off:512],
                                    start=True,
                                    stop=True,
                                )
                                if kb >= 4 * qt:
                                    nc.vector.tensor_tensor(
                                        pr[:, 512 * s + off : 512 * s + off + 128],
                                        pr[:, 512 * s + off : 512 * s + off + 128],
                                        cap_t[:],
                                        mybir.AluOpType.min,
                                    )
                            ex = wpool.tile([128, 1024], BF16, name="ex", tag="ex", bufs=5)
                            nc.scalar.activation(
                                ex[:, offs[0] : 1024],
                                pr[:, offs[0] : 1024],
                                mybir.ActivationFunctionType.Exp,
                                bias=kbias_t[:, hl : hl + 1],
                                scale=0.125,
                            )
                            pend.append((pairs[pi], offs, ex))
                        if pi >= 1:
                            (kb0, kb1), offs, ex = pend.pop(0)
                            for s, (kb, off) in enumerate(zip((kb0, kb1), offs)):
                                nc.tensor.matmul(
                                    ot[:, off:512],
                                    vt[b][kb][:, 65 * hl : 65 * hl + 65],
                                    ex[:, 512 * s + off : 512 * (s + 1)],
                                    start=(kb == kept[0]),
                                    stop=(kb == kept[-1]),
                                )
                # Copy OT out of PSUM immediately (one op per head, split
                # across ScalarE/VectorE) so the psot slots release for the
                # next group; the whole normalize chain then runs from SBUF
                # off the inter-group critical path.
                otf0 = wpool.tile([65, 512], F32, name="otf0", tag="otf0", bufs=2)
                otf1 = wpool.tile([128, 512], F32, name="otf1", tag="otf1", bufs=2)
                denb = wpool.tile([1, 512], F32, name="denb", tag="denb", bufs=2)
                nc.scalar.copy(otf0[:], ots[0][:])
                nc.vector.tensor_copy(otf1[64:128, :], ots[1][0:64, :])
                nc.vector.tensor_copy(denb[:], ots[1][64:65, :])
                den2 = wpool.tile([1, 1024], F32, name="den2", tag="den2", bufs=2)
                nc.vector.tensor_copy(den2[:, 0:512], otf0[64:65, :])
                nc.vector.tensor_copy(den2[:, 512:1024], denb[:])
                bcs = wpool.tile([128, 1024], F32, name="bcs", tag="bcs", bufs=2)
                nc.gpsimd.partition_broadcast(bcs[:], den2[:])
                bci = wpool.tile([128, 1024], F32, name="bci", tag="bci", bufs=2)
                nc.vector.reciprocal_approx_fast(bci[:], bcs[:])
                otn = wpool.tile([128, 512], BF16, name="otn", tag="otn", bufs=4)
                nc.vector.tensor_tensor(
                    otn[0:64, :], otf0[0:64, :], bci[0:64, 0:512], mybir.AluOpType.mult
                )
                nc.vector.tensor_tensor(
                    otn[64:128, :], otf1[64:128, :], bci[64:128, 512:1024], mybir.AluOpType.mult
                )
                # two destination blocks of 256 tokens each
                for half in range(2):
                    blk = 2 * qt + half
                    nc.sync.dma_start(
                        ccin[b][128 * blk : 128 * (blk + 1), :],
                        otn[:, 256 * half : 256 * (half + 1)],
                    )

            # ---- phase 4: output projection for one batch -------------
            at = [[None] * 8 for _ in range(B)]

            def yrecv(b):
                for k in range(8):
                    a_ = cpool.tile([128, TPC // B], BF16, name=f"at{b}_{k}", tag=f"at{b}_{k}")
                    nc.sync.dma_start(a_[:], ccout[b][128 * k : 128 * (k + 1), :])
                    at[b][k] = a_

            def ypiece(b, tb, n):
                yp = ps.tile([128, 512], F32, name="yp", tag="mm512")
                for k in range(8):
                    nc.tensor.matmul(
                        yp[:],
                        at[b][k][:, 128 * tb : 128 * (tb + 1)],
                        wo_t[:, D * k + 512 * n : D * k + 512 * (n + 1)],
                        start=(k == 0),
                        stop=(k == 7),
                    )
                ys = wpool.tile([128, 512], F32, name="ys", tag="ys", bufs=2)
                nc.vector.tensor_copy(ys[:], yp[:])
                nc.sync.dma_start(
                    out_d[256 * b + 128 * tb : 256 * b + 128 * (tb + 1), 512 * n : 512 * (n + 1)],
                    ys[:],
                )

            # ---- schedule -------------------------------------------
            for qt in range(NQT):
                qkv_chunk(qt)
                attn_group(0, qt)
            # wo arrives during attention on the sync queue
            wo_t = cpool.tile([128, 8 * D], BF16, name="wo_t", tag="wo_t")
            for k in range(8):
                nc.sync.dma_start(wo_t[:, D * k : D * (k + 1)], wo_d[128 * k : 128 * (k + 1), :])
            nc.gpsimd.collective_compute(
                "AllToAll",
                mybir.AluOpType.bypass,
                replica_groups=[list(range(NC))],
                ins=[ccin[0][:]],
                outs=[ccout[0][:]],
            )
            yrecv(0)
            for qt in range(NQT):
                qkv_chunk(NQT + qt)
                attn_group(1, qt)
            for tb in range(2):
                for n in range(D // 512):
                    ypiece(0, tb, n)
            nc.gpsimd.collective_compute(
                "AllToAll",
                mybir.AluOpType.bypass,
                replica_groups=[list(range(NC))],
                ins=[ccin[1][:]],
                outs=[ccout[1][:]],
            )
            yrecv(1)
            for tb in range(2):
                for n in range(D // 512):
                    ypiece(1, tb, n)

    nc.compile()
    return nc


def _host_inputs(x, Wq, Wk, Wv, Wo):
    x = np.asarray(x, dtype=np.float32)
    Wq, Wk, Wv, Wo = (np.asarray(w, dtype=np.float32) for w in (Wq, Wk, Wv, Wo))
    toks = x.reshape(TOK, D)
    xT = np.ascontiguousarray(toks.T).astype(NPBF16)
    base = 2.0 ** (-8.0 / H)

    cap = np.where(
        np.arange(128)[:, None] <= np.arange(128)[None, :], 3.0e38, -1.0e9
    ).astype(np.float32)
    ind = np.zeros((1, 256), dtype=np.float32)
    ind[0, 0:64] = 1      # head-A indicator: bc rows 0:64 get denA
    ind[0, 192:256] = 1   # head-B indicator: bc rows 64:128 get denB
    pos = np.arange(T, dtype=np.float32)
    pos_bf = pos.astype(NPBF16).astype(np.float32)
    kbrow = np.floor(pos / 128.0).astype(NPBF16)  # k-block index, exact
    ones_row = np.ones(T, dtype=NPBF16)
    kaug = np.stack([ones_row, kbrow, kbrow])  # rows 64..66 of K'

    in_maps = []
    for c in range(NC):
        heads = [c, 8 + c]  # steep slot A, shallow slot B
        rows = np.concatenate([np.arange(64 * g, 64 * (g + 1)) for g in heads])
        qaug = np.zeros((HPC * 3, T), dtype=NPBF16)
        kbias = np.zeros((128, HPC), dtype=np.float32)
        for hl, g in enumerate(heads):
            slope = float(base ** (g + 1))
            qaug[3 * hl + 0] = (-8.0 * slope * pos_bf).astype(NPBF16)
            cc = NPBF16(1024.0 * slope)
            dr = NPBF16(1024.0 * slope - float(cc))
            qaug[3 * hl + 1] = cc
            qaug[3 * hl + 2] = dr
            kbias[:, hl] = slope * np.arange(128)
        in_maps.append(
            {
                "xT": xT,
                "wqkv": np.ascontiguousarray(
                    np.concatenate(
                        [Wq[rows, :].T, Wk[rows, :].T, Wv[rows, :].T], axis=1
                    )
                ).astype(NPBF16),
                "wo": None,  # filled below (same for all cores)
                "qaug": qaug,
                "kaug": kaug,
                "kbias": kbias,
                "cap": cap,
                "ind": ind,
            }
        )
    # Wo rows permuted to match the concat order the a2a produces:
    # source core p contributes [head p dims ; head 8+p dims].
    perm = np.concatenate(
        [
            np.concatenate(
                [np.arange(64 * p, 64 * (p + 1)), np.arange(64 * (8 + p), 64 * (9 + p))]
            )
            for p in range(NC)
        ]
    )
    wo_t = np.ascontiguousarray(Wo.T[perm, :]).astype(NPBF16)
    for m in in_maps:
        m["wo"] = wo_t
    return in_maps


def get_compiled():
    global _COMPILED
    if _COMPILED is None:
        _COMPILED = _build()
    return _COMPILED


def run(x, Wq, Wk, Wv, Wo, trace=False, **trace_kwargs):
    nc = get_compiled()
    in_maps = _host_inputs(x, Wq, Wk, Wv, Wo)
    res = bass_utils.run_bass_kernel_spmd(
        nc, in_maps, core_ids=list(range(NC)), trace=trace, **trace_kwargs
    )
    full = np.empty((TOK, D), dtype=np.float32)
    half = TPC // B  # 256
    for c in range(NC):
        o = res.results[c]["out"]
        full[half * c : half * (c + 1), :] = o[0:half]
        full[T + half * c : T + half * (c + 1), :] = o[half : 2 * half]
    return full.reshape(B, T, D), res


def kernel(x, Wq, Wk, Wv, Wo):
    out, _ = run(x, Wq, Wk, Wv, Wo)
    return out
